# revision 43
# baseline (speedup 1.0000x reference)
"""Trainium2 Bass kernel for a spatial self-attention block.

Reference computation (per batch element b):
    q = w1 @ x + b1   [32, HW]      (1x1 conv == channel-wise linear)
    k = w2 @ x + b2   [32, HW]
    v = w3 @ x + b3   [256, HW]
    e[i, j] = sum_c q[c, i] k[c, j]
    attn = softmax(e, axis=j)
    out[c, i] = sum_j v[c, j] attn[i, j] + x[c, i]

Sharding: batch (8) across the 8 NeuronCores, one image per core.

v2 design (vs the v1 pair-staggered kernel; ~131us vs v1's ~143us at
the full-clock DVFS epoch; the shared host also has a ~1.2x-slower
epoch, run-level, visible as AV matmul dur 327ns vs 273ns):
  * Sweeps of 256 queries (16 sweeps). Each j-quad (4 key tiles x 256
    queries) is 4 pack matmuls into ONE [128,4,256] 2-bank PSUM tile
    consumed by ONE exp slot (FD=1024) - a single release event per
    quad. PSUM: 3 quad tiles (6 banks) + 2 AV accumulators = 8 banks.
  * HW constraint found by bisection: two CONCURRENT pack strips
    streaming into the same PSUM bank wedge the device (redacted
    INTERNAL at runtime; CoreSim doesn't model it). Strips sharing a
    bank therefore share a PE row-group (the row band serializes
    them); bank leaders run 2-wide, and alternating quads use the
    other row-group pair so adjacent quads can overlap 4-wide.
  * Measured transition tax (microbench, dtype/FWL-invariant):
    full-array<->pack costs ~197ns of PE pipeline drain per switch;
    strip->strip issue is ~3ns. Quads are therefore grouped in trains
    of {3,3,2} per sweep ahead of AV blocks of {16,12,4}: the longer
    first block lets the exp engines drain so the next train's ep
    buffers release together and the train actually forms. Mid-sweep
    the ep-release pacing (one FD1024 exp per ~585ns across ACT+DVE)
    still degrades some trains to solo quads - structural with 3 ep
    buffers; 4 don't fit in PSUM.
  * Exp slots strictly alternate DVE (Schraudolph fast-exp
    tensor_scalar into a u8-viewed tile) / ACT (real Exp, fp8e4 out).
    Both are 1x-capped by the f32 PSUM read port - the exp wall is
    ~90us/engine including evacs+flushes and is the co-floor with the
    PE (~116us busy).
  * Evacuations batched to amortize the ~150-230ns per-op fixed cost:
    v FD=1024 (one per 512-col chunk), k FD=1024 (per chunk pair),
    q FD=512 (per sweep pair). Input DMAs split across the sync and
    gpsimd queues so the phase-1 load pipeline is ~2x wider.
  * AV: fp8 DoubleRow, stationary = pt strips (K=256 keys), moving =
    vt[.,257] (v channels | ones column -> softmax denominator),
    N=257 at ~110ns issue rate (1 col/cycle @2.4GHz; DR doubles K per
    instruction, not the column rate); raw accumulator flushed bf16
    to HBM, normalization + residual on host (error budget: attention
    output is ~170x smaller in norm than the residual).
  * Weights pre-scaled by 64 (pow2) for fp8 range; 1/64 folded into
    the PSUM evacuations. End-to-end rel err ~7e-4 (gate 2e-2).
"""

import numpy as np
import ml_dtypes

B, C, H, W = 8, 256, 64, 64
HW = H * W          # 4096
CQK = C // 8        # 32
NCORES = 8
NJ = HW // 128      # 32 key tiles
ICH = 16            # query-dim chunks (sweeps)
CHUNK = HW // ICH   # 256 queries per sweep
NQ = NJ // 4        # 8 quads of key tiles per sweep
VSTRIDE = 260       # vT free-dim stride per j-tile (257 used, 260 align)
WSCALE = 64.0       # host-side pow2 weight prescale for fp8 range

# Schraudolph fast-exp constants (e4m3 bit pattern): bits = A8*e + B8
A8 = 8.0 * 1.4426950408889634   # 8 * log2(e)
B8 = 56.0 - 0.45                # (bias 7)*8 minus tuned PWL offset

_cache: dict = {}


def _build_program():
    import concourse.bacc as bacc
    import concourse.mybir as mybir
    import concourse.tile as tile

    f32 = mybir.dt.float32
    bf16 = mybir.dt.bfloat16
    fp8 = mybir.dt.float8e4
    u8 = mybir.dt.uint8
    Exp = mybir.ActivationFunctionType.Exp
    Identity = mybir.ActivationFunctionType.Identity
    Copy = mybir.ActivationFunctionType.Copy
    DR = mybir.MatmulPerfMode.DoubleRow
    MUL = mybir.AluOpType.mult
    ADD = mybir.AluOpType.add

    nc = bacc.Bacc(None)
    x8_d = nc.dram_tensor("x8", [128, 2, HW], fp8, kind="ExternalInput")
    w1_d = nc.dram_tensor("w1t4", [128, 2, 128], fp8, kind="ExternalInput")
    w2_d = nc.dram_tensor("w2t4", [128, 2, 128], fp8, kind="ExternalInput")
    w3_d = nc.dram_tensor("w3t", [128, 2, C], fp8, kind="ExternalInput")
    b1_d = nc.dram_tensor("b1r4", [128, 1], f32, kind="ExternalInput")
    b2_d = nc.dram_tensor("b2r4", [128, 1], f32, kind="ExternalInput")
    # raw AV accumulator (numerator | denominator): normalization and the
    # +x residual happen on the host in fp32
    outt_d = nc.dram_tensor("outt", [HW, 257], bf16, kind="ExternalOutput")

    with tile.TileContext(nc) as tc:
        with (
            tc.tile_pool(name="const", bufs=1) as cpool,
            tc.tile_pool(name="xin", bufs=1) as xpool,
            tc.tile_pool(name="qk", bufs=1) as qkpool,
            tc.tile_pool(name="pt", bufs=18) as ptpool,
            tc.tile_pool(name="io", bufs=3) as iopool,
            tc.tile_pool(name="psume", bufs=3, space="PSUM") as epool,
            tc.tile_pool(name="psumo", bufs=2, space="PSUM") as opool,
        ):
            # ---- constants / weights ----
            w1t4 = cpool.tile([128, 2, 128], fp8, tag="w1t4", name="w1t4")
            w2t4 = cpool.tile([128, 2, 128], fp8, tag="w2t4", name="w2t4")
            w3t = cpool.tile([128, 2, C], fp8, tag="w3t", name="w3t")
            b1r4 = cpool.tile([128, 1], f32, tag="b1r4", name="b1r4")
            b2r4 = cpool.tile([128, 1], f32, tag="b2r4", name="b2r4")
            x8 = xpool.tile([128, 2, HW], fp8, tag="x8", name="x8")
            # Inputs split across the sync and (phase-1-idle) gpsimd DMA
            # queues so the load pipeline is ~2x wider; x chunk 0 + the
            # small qk weights lead so the first kproj can start ASAP.
            nc.sync.dma_start(w2t4[:], w2_d[:])
            nc.sync.dma_start(x8[:, :, 0:256], x8_d[:, :, 0:256])
            nc.sync.dma_start(x8[:, :, 256:512], x8_d[:, :, 256:512])
            nc.sync.dma_start(x8[:, :, 512:1024], x8_d[:, :, 512:1024])
            nc.sync.dma_start(w1t4[:], w1_d[:])
            nc.sync.dma_start(b2r4[:], b2_d[:])
            nc.sync.dma_start(b1r4[:], b1_d[:])
            nc.sync.dma_start(w3t[:], w3_d[:])
            for g in range(2, 8):
                cs = slice(g * 512, (g + 1) * 512)
                nc.gpsimd.dma_start(x8[:, :, cs], x8_d[:, :, cs])

            warm = cpool.tile([128, 512], bf16, tag="warm", name="warm")
            nc.vector.memset(warm[:], 0.0)

            # q and k live replicated 4x along the partition axis (copies
            # at base partitions 0/32/64/96) so the eT pack matmuls can use
            # all 128 PE rows. vt[j, c] holds v (plus a ones column at 256
            # that makes the softmax denominator fall out of the AV pass).
            q_sb = qkpool.tile([128, ICH, CHUNK], bf16, tag="q", name="q")
            k_sb = qkpool.tile([128, ICH, CHUNK], bf16, tag="k", name="k")
            vt = qkpool.tile([128, NJ, VSTRIDE], fp8, tag="vt", name="vt")
            nc.vector.memset(vt[:, :, 256:257], 1.0)

            def warm_mm(dst, n=1):
                # dummy full-array matmul into a psum region that a real
                # matmul overwrites later (start=True resets it); trips the
                # HAM activity monitor so the PE clock ramps to 2.4 GHz
                for _ in range(n):
                    nc.tensor.matmul(dst, warm[:, 0:128], warm[:, 0:256],
                                     start=True, stop=True)

            # global exp-slot parity: strict ACT/DVE alternation
            state = {"slot": 0, "nflush": 0}
            pt_handles = [[None] * NQ for _ in range(ICH)]

            def emit_exp(ep, s, u):
                pt = ptpool.tile([128, 4, CHUNK], fp8, tag="pt", name="pt")
                if state["slot"] % 2 == 0:
                    nc.vector.tensor_scalar(pt.bitcast(u8)[:], ep[:],
                                            A8, B8, MUL, ADD)
                else:
                    nc.scalar.activation(pt[:], ep[:], Exp)
                state["slot"] += 1
                pt_handles[s][u] = pt

            def emit_quad(s, u):
                # 4 pack matmuls -> one 2-bank tile. HW constraint (found
                # by bisection): two CONCURRENT pack strips into the same
                # PSUM bank wedge the device. Strips sharing a bank
                # therefore share a PE row-group (the row band serializes
                # them); the two banks' leaders run concurrently, and
                # alternating quads use the other row-group pair so
                # consecutive quads can overlap 4-wide.
                ep = epool.tile([128, 4, CHUNK], f32, tag="e", name="e")
                pb = (u % 2) * 2
                for i in (0, 2, 1, 3):   # bank leaders first, then trailers
                    jt = 4 * u + i
                    t = pb + i // 2
                    nc.tensor.matmul(
                        ep[:, i, :],
                        k_sb[t * CQK:(t + 1) * CQK, jt // 2,
                             (jt % 2) * 128:(jt % 2) * 128 + 128],
                        q_sb[t * CQK:(t + 1) * CQK, s, :],
                        start=True, stop=True,
                        tile_position=(t * CQK, 0))
                emit_exp(ep, s, u)

            def emit_qproj(s):
                # q columns for sweeps s, s+1 (512 cols), one evac on ACT
                ep_q = epool.tile([128, 4, CHUNK], f32, tag="e", name="e")
                for i in range(2):
                    cs = slice((s + i) * CHUNK, (s + i + 1) * CHUNK)
                    nc.tensor.matmul(ep_q[:, i, :], w1t4[:], x8[:, :, cs],
                                     start=True, stop=True, perf_mode=DR)
                nc.scalar.activation(
                    q_sb[:, s:s + 2, :], ep_q[:, 0:2, :],
                    Identity, bias=b1r4[:], scale=1.0 / WSCALE)

            wep = epool.tile([128, 4, CHUNK], f32, tag="e", name="e")
            warm_mm(wep[:, 0, :], 2)

            def emit_phase1_gp(gp):
                # k projections for chunks 2gp, 2gp+1 -> one FD1024 evac
                ep_k = epool.tile([128, 4, CHUNK], f32, tag="e", name="e")
                for i in range(4):
                    cs = slice((4 * gp + i) * CHUNK,
                               (4 * gp + i + 1) * CHUNK)
                    nc.tensor.matmul(ep_k[:, i, :], w2t4[:], x8[:, :, cs],
                                     start=True, stop=True, perf_mode=DR)
                # phase 1 is DVE-critical (exps + evacs): k evacs split
                # 2/2 and v evacs 5/3 toward the cheaper-per-op ACT
                if gp % 2 == 0:
                    nc.vector.tensor_scalar(
                        k_sb[:, 4 * gp:4 * gp + 4, :], ep_k[:],
                        1.0 / WSCALE, b2r4[:], MUL, ADD)
                else:
                    nc.scalar.activation(
                        k_sb[:, 4 * gp:4 * gp + 4, :], ep_k[:],
                        Identity, bias=b2r4[:], scale=1.0 / WSCALE)
                for i in range(2):
                    g = 2 * gp + i
                    # v for the 4 j-tiles of chunk g: one FD1024 evac
                    ep_v = epool.tile([128, 4, CHUNK], f32, tag="e",
                                      name="e")
                    for jj in range(4):
                        j = 4 * g + jj
                        nc.tensor.matmul(
                            ep_v[:, jj, :],
                            x8[:, :, j * 128:(j + 1) * 128], w3t[:],
                            start=True, stop=True, perf_mode=DR)
                    dst = vt[:, 4 * g:4 * g + 4, 0:C]
                    if g % 2 == 0 or g == 7:
                        nc.scalar.activation(dst, ep_v[:], Copy,
                                             scale=1.0 / WSCALE)
                    else:
                        nc.vector.tensor_scalar(dst, ep_v[:], 1.0 / WSCALE,
                                                None, MUL)
                    # sweep-0 quad for key tiles 4g..4g+3 vs query chunk 0
                    emit_quad(0, g)

            # ---- sweeps 1..16: per u (j-quad): one quad + exp for sweep
            # s, and 4 AV matmuls for sweep s-1. The AVs of u 0..3 cover
            # i-tile 0, u 4..7 i-tile 1 (staggered po lifetimes so the
            # 2-buf po pool never reallocates before its flush is
            # emitted). AV first: its pt inputs are a sweep old and
            # always ready, so the PE covers the quad's ep-buffer wait
            # with useful work.
            po = None
            pending = None     # (po, i0) awaiting evacuation; deferred to
            # the next it-group so the copy never head-of-line-blocks the
            # exp stream on either engine queue

            def flush_pending():
                nonlocal pending
                if pending is None:
                    return
                fpo, fi0 = pending
                ot = iopool.tile([128, 257], bf16, tag="ot", name="ot")
                if state["nflush"] % 2 == 0:
                    nc.scalar.activation(ot[:], fpo[:], Copy)
                else:
                    nc.vector.tensor_copy(ot[:], fpo[:])
                state["nflush"] += 1
                nc.gpsimd.dma_start(outt_d[fi0:fi0 + 128, :], ot[:])
                pending = None

            def emit_av(bs, it, kk):
                up, p = kk // 2, kk % 2
                nc.tensor.matmul(
                    po[:],
                    pt_handles[bs][up][:, 2 * p:2 * p + 2,
                                       it * 128:(it + 1) * 128],
                    vt[:, 2 * kk:2 * kk + 2, 0:257],
                    start=(kk == 0), stop=(kk == 15),
                    perf_mode=DR)

            # Pairing two quads per AV block halves the full-array<->pack
            # mode switches (each costs ~200ns of PE pipeline drain); the
            # two quads of a pair use different row-group sets AND banks,
            # so their strips overlap up to 4-wide.
            acnt = {}

            def emit_av_block(bs, n, qp=None):
                nonlocal po, pending
                for _ in range(n):
                    a = acnt.get(bs, 0)
                    acnt[bs] = a + 1
                    it, kk = a // 16, a % 16
                    if kk == 0:
                        flush_pending()
                        po = opool.tile([128, 257], f32, tag="o",
                                        name="o")
                        if it == 0 and qp is not None:
                            emit_qproj(qp)
                    emit_av(bs, it, kk)
                    if kk == 15:
                        pending = (po, bs * CHUNK + it * 128)

            # ---- phase 1: projections + sweep-0 quads, then sweeps.
            emit_qproj(0)
            for gp in range(4):
                emit_phase1_gp(gp)

            # ---- sweeps 1..16. Quads are grouped into trains of {3,3,2}
            # per sweep: each full-array<->pack transition costs ~197ns of
            # PE pipeline drain (row-conflict, measured; dtype/FWL-
            # invariant) while strip->strip issue inside a train is ~3ns,
            # so 3 transitions per sweep beat 8. Trains go BEFORE their AV
            # blocks so the exps enqueue early and ep buffers recycle with
            # slack.
            groups = [(3, 16), (3, 12), (2, 4)]
            for s in range(1, ICH + 1):
                bs = s - 1
                qbase = 0
                for gi, (nq, nav) in enumerate(groups):
                    if s < ICH:
                        for j in range(nq):
                            emit_quad(s, qbase + j)
                        qbase += nq
                    qp = None
                    if gi == 0 and s % 2 == 1 and s + 2 < ICH:
                        qp = s + 1
                    emit_av_block(bs, nav, qp=qp)
            flush_pending()

    nc.compile()
    return nc


def _get_program():
    if "nc" not in _cache:
        _cache["nc"] = _build_program()
    return _cache["nc"]


def _in_maps(inputs: dict) -> list:
    e4 = ml_dtypes.float8_e4m3
    x = np.asarray(inputs["x"], np.float32)
    w1 = np.asarray(inputs["w1"], np.float32)
    w2 = np.asarray(inputs["w2"], np.float32)
    w3 = np.asarray(inputs["w3"], np.float32)
    b1 = np.asarray(inputs["b1"], np.float32)
    b2 = np.asarray(inputs["b2"], np.float32)

    def rep4(w):  # [32, 256] -> [128, 2, 128] stationary, out cols tiled 4x
        wr = np.tile(w * WSCALE, (4, 1))                     # [128, 256]
        return np.ascontiguousarray(
            wr.T.reshape(2, 128, 128).transpose(1, 0, 2)).astype(e4)

    w1t4 = rep4(w1)
    w2t4 = rep4(w2)
    w3t8 = np.ascontiguousarray(
        (w3 * WSCALE).T.reshape(2, 128, C).transpose(1, 0, 2)).astype(e4)
    b1r4 = np.tile(b1, 4)[:, None].astype(np.float32)
    b2r4 = np.tile(b2, 4)[:, None].astype(np.float32)
    maps = []
    for b in range(B):
        xb = x[b].reshape(C, HW)
        x8 = np.ascontiguousarray(
            xb.reshape(2, 128, HW).transpose(1, 0, 2)).astype(e4)
        maps.append({
            "x8": x8,
            "w1t4": w1t4, "w2t4": w2t4, "w3t": w3t8,
            "b1r4": b1r4, "b2r4": b2r4,
        })
    return maps


def kernel(**inputs) -> np.ndarray:
    from concourse.bass_utils import run_bass_kernel_spmd

    nc = _get_program()
    res = run_bass_kernel_spmd(nc, _in_maps(inputs), list(range(NCORES)))
    x = np.asarray(inputs["x"], np.float32)
    b3 = np.asarray(inputs["b3"], np.float32)
    out = np.empty((B, C, H, W), np.float32)
    for b in range(B):
        acc = res.results[b]["outt"].astype(np.float32)   # [HW, 257]
        attn_t = acc[:, 0:256] / acc[:, 256:257] + b3[None, :]
        out[b] = attn_t.T.reshape(C, H, W) + x[b]
    return out


# revision 44
# speedup vs baseline: 1.1724x; 1.1724x over previous
"""Trainium2 Bass kernel for a spatial self-attention block.

Reference computation (per batch element b):
    q = w1 @ x + b1   [32, HW]      (1x1 conv == channel-wise linear)
    k = w2 @ x + b2   [32, HW]
    v = w3 @ x + b3   [256, HW]
    e[i, j] = sum_c q[c, i] k[c, j]
    attn = softmax(e, axis=j)
    out[c, i] = sum_j v[c, j] attn[i, j] + x[c, i]

Sharding: batch (8) across the 8 NeuronCores, one image per core.

v2 design (vs the v1 pair-staggered kernel; ~131us vs v1's ~143us at
the full-clock DVFS epoch; the shared host also has a ~1.2x-slower
epoch, run-level, visible as AV matmul dur 327ns vs 273ns):
  * Sweeps of 256 queries (16 sweeps). Each j-quad (4 key tiles x 256
    queries) is 4 pack matmuls into ONE [128,4,256] 2-bank PSUM tile
    consumed by ONE exp slot (FD=1024) - a single release event per
    quad. PSUM: 3 quad tiles (6 banks) + 2 AV accumulators = 8 banks.
  * HW constraint found by bisection: two CONCURRENT pack strips
    streaming into the same PSUM bank wedge the device (redacted
    INTERNAL at runtime; CoreSim doesn't model it). Strips sharing a
    bank therefore share a PE row-group (the row band serializes
    them); bank leaders run 2-wide, and alternating quads use the
    other row-group pair so adjacent quads can overlap 4-wide.
  * Measured transition tax (microbench, dtype/FWL-invariant):
    full-array<->pack costs ~197ns of PE pipeline drain per switch;
    strip->strip issue is ~3ns. Quads are therefore grouped in trains
    of {3,3,2} per sweep ahead of AV blocks of {16,12,4}: the longer
    first block lets the exp engines drain so the next train's ep
    buffers release together and the train actually forms. Mid-sweep
    the ep-release pacing (one FD1024 exp per ~585ns across ACT+DVE)
    still degrades some trains to solo quads - structural with 3 ep
    buffers; 4 don't fit in PSUM.
  * Exp slots strictly alternate DVE (Schraudolph fast-exp
    tensor_scalar into a u8-viewed tile) / ACT (real Exp, fp8e4 out).
    Both are 1x-capped by the f32 PSUM read port - the exp wall is
    ~90us/engine including evacs+flushes and is the co-floor with the
    PE (~116us busy).
  * Evacuations batched to amortize the ~150-230ns per-op fixed cost:
    v FD=1024 (one per 512-col chunk), k FD=1024 (per chunk pair),
    q FD=512 (per sweep pair). Input DMAs split across the sync and
    gpsimd queues so the phase-1 load pipeline is ~2x wider.
  * AV: fp8 DoubleRow, stationary = pt strips (K=256 keys), moving =
    vt[.,257] (v channels | ones column -> softmax denominator),
    N=257 at ~110ns issue rate (1 col/cycle @2.4GHz; DR doubles K per
    instruction, not the column rate); raw accumulator flushed bf16
    to HBM, normalization + residual on host (error budget: attention
    output is ~170x smaller in norm than the residual).
  * Weights pre-scaled by 64 (pow2) for fp8 range; 1/64 folded into
    the PSUM evacuations. End-to-end rel err ~7e-4 (gate 2e-2).
"""

import numpy as np
import ml_dtypes

B, C, H, W = 8, 256, 64, 64
HW = H * W          # 4096
CQK = C // 8        # 32
NCORES = 8
NJ = HW // 128      # 32 key tiles
ICH = 16            # query-dim chunks (sweeps)
CHUNK = HW // ICH   # 256 queries per sweep
NQ = NJ // 4        # 8 quads of key tiles per sweep
VSTRIDE = 260       # vT free-dim stride per j-tile (257 used, 260 align)
WSCALE = 64.0       # host-side pow2 weight prescale for fp8 range

# Schraudolph fast-exp constants (e4m3 bit pattern): bits = A8*e + B8
A8 = 8.0 * 1.4426950408889634   # 8 * log2(e)
B8 = 56.0 - 0.45                # (bias 7)*8 minus tuned PWL offset

_cache: dict = {}


def _build_program():
    import concourse.bacc as bacc
    import concourse.mybir as mybir
    import concourse.tile as tile

    f32 = mybir.dt.float32
    bf16 = mybir.dt.bfloat16
    fp8 = mybir.dt.float8e4
    u8 = mybir.dt.uint8
    Exp = mybir.ActivationFunctionType.Exp
    Identity = mybir.ActivationFunctionType.Identity
    Copy = mybir.ActivationFunctionType.Copy
    DR = mybir.MatmulPerfMode.DoubleRow
    MUL = mybir.AluOpType.mult
    ADD = mybir.AluOpType.add

    nc = bacc.Bacc(None)
    x8_d = nc.dram_tensor("x8", [128, 2, HW], fp8, kind="ExternalInput")
    w1_d = nc.dram_tensor("w1t4", [128, 2, 128], fp8, kind="ExternalInput")
    w2_d = nc.dram_tensor("w2t4", [128, 2, 128], fp8, kind="ExternalInput")
    w3_d = nc.dram_tensor("w3t", [128, 2, C], fp8, kind="ExternalInput")
    b1_d = nc.dram_tensor("b1r4", [128, 1], f32, kind="ExternalInput")
    b2_d = nc.dram_tensor("b2r4", [128, 1], f32, kind="ExternalInput")
    # raw AV accumulator (numerator | denominator): normalization and the
    # +x residual happen on the host in fp32
    outt_d = nc.dram_tensor("outt", [HW, 257], bf16, kind="ExternalOutput")

    with tile.TileContext(nc) as tc:
        with (
            tc.tile_pool(name="const", bufs=1) as cpool,
            tc.tile_pool(name="xin", bufs=1) as xpool,
            tc.tile_pool(name="qk", bufs=1) as qkpool,
            tc.tile_pool(name="pt", bufs=18) as ptpool,
            tc.tile_pool(name="io", bufs=3) as iopool,
            tc.tile_pool(name="psume", bufs=3, space="PSUM") as epool,
            tc.tile_pool(name="psumo", bufs=2, space="PSUM") as opool,
        ):
            # ---- constants / weights ----
            w1t4 = cpool.tile([128, 2, 128], fp8, tag="w1t4", name="w1t4")
            w2t4 = cpool.tile([128, 2, 128], fp8, tag="w2t4", name="w2t4")
            w3t = cpool.tile([128, 2, C], fp8, tag="w3t", name="w3t")
            b1r4 = cpool.tile([128, 1], f32, tag="b1r4", name="b1r4")
            b2r4 = cpool.tile([128, 1], f32, tag="b2r4", name="b2r4")
            x8 = xpool.tile([128, 2, HW], fp8, tag="x8", name="x8")
            # Inputs split across the sync and (phase-1-idle) gpsimd DMA
            # queues so the load pipeline is ~2x wider; x chunk 0 + the
            # small qk weights lead so the first kproj can start ASAP.
            nc.sync.dma_start(w2t4[:], w2_d[:])
            nc.sync.dma_start(x8[:, :, 0:256], x8_d[:, :, 0:256])
            nc.sync.dma_start(x8[:, :, 256:512], x8_d[:, :, 256:512])
            nc.sync.dma_start(w1t4[:], w1_d[:])
            nc.sync.dma_start(x8[:, :, 512:1024], x8_d[:, :, 512:1024])
            nc.sync.dma_start(b2r4[:], b2_d[:])
            nc.sync.dma_start(b1r4[:], b1_d[:])
            nc.sync.dma_start(w3t[:], w3_d[:])
            for g in range(2, 8):
                cs = slice(g * 512, (g + 1) * 512)
                nc.gpsimd.dma_start(x8[:, :, cs], x8_d[:, :, cs])

            warm = cpool.tile([128, 512], bf16, tag="warm", name="warm")
            nc.vector.memset(warm[:], 0.0)

            # q and k live replicated 4x along the partition axis (copies
            # at base partitions 0/32/64/96) so the eT pack matmuls can use
            # all 128 PE rows. vt[j, c] holds v (plus a ones column at 256
            # that makes the softmax denominator fall out of the AV pass).
            q_sb = qkpool.tile([128, ICH, CHUNK], bf16, tag="q", name="q")
            k_sb = qkpool.tile([128, ICH, CHUNK], bf16, tag="k", name="k")
            vt = qkpool.tile([128, NJ, VSTRIDE], fp8, tag="vt", name="vt")
            nc.vector.memset(vt[:, :, 256:257], 1.0)

            def warm_mm(dst, n=1):
                # dummy full-array matmul into a psum region that a real
                # matmul overwrites later (start=True resets it); trips the
                # HAM activity monitor so the PE clock ramps to 2.4 GHz
                for _ in range(n):
                    nc.tensor.matmul(dst, warm[:, 0:128], warm[:, 0:256],
                                     start=True, stop=True)

            # global exp-slot parity: strict ACT/DVE alternation
            state = {"slot": 0, "nflush": 0}
            pt_handles = [[None] * NQ for _ in range(ICH)]

            def emit_exp(ep, s, u):
                pt = ptpool.tile([128, 4, CHUNK], fp8, tag="pt", name="pt")
                if state["slot"] % 2 == 0:
                    nc.vector.tensor_scalar(pt.bitcast(u8)[:], ep[:],
                                            A8, B8, MUL, ADD)
                else:
                    nc.scalar.activation(pt[:], ep[:], Exp)
                state["slot"] += 1
                pt_handles[s][u] = pt

            def emit_quad(s, u):
                # 4 pack matmuls -> one 2-bank tile. HW constraint (found
                # by bisection): two CONCURRENT pack strips into the same
                # PSUM bank wedge the device. Strips sharing a bank
                # therefore share a PE row-group (the row band serializes
                # them); the two banks' leaders run concurrently, and
                # alternating quads use the other row-group pair so
                # consecutive quads can overlap 4-wide.
                ep = epool.tile([128, 4, CHUNK], f32, tag="e", name="e")
                pb = (u % 2) * 2
                for i in (0, 2, 1, 3):   # bank leaders first, then trailers
                    jt = 4 * u + i
                    t = pb + i // 2
                    nc.tensor.matmul(
                        ep[:, i, :],
                        k_sb[t * CQK:(t + 1) * CQK, jt // 2,
                             (jt % 2) * 128:(jt % 2) * 128 + 128],
                        q_sb[t * CQK:(t + 1) * CQK, s, :],
                        start=True, stop=True,
                        tile_position=(t * CQK, 0))
                emit_exp(ep, s, u)

            def emit_qproj(s):
                # q columns for sweeps s, s+1 (512 cols), one evac on ACT
                ep_q = epool.tile([128, 4, CHUNK], f32, tag="e", name="e")
                for i in range(2):
                    cs = slice((s + i) * CHUNK, (s + i + 1) * CHUNK)
                    nc.tensor.matmul(ep_q[:, i, :], w1t4[:], x8[:, :, cs],
                                     start=True, stop=True, perf_mode=DR)
                nc.scalar.activation(
                    q_sb[:, s:s + 2, :], ep_q[:, 0:2, :],
                    Identity, bias=b1r4[:], scale=1.0 / WSCALE)

            wep = epool.tile([128, 4, CHUNK], f32, tag="e", name="e")
            warm_mm(wep[:, 0, :], 2)

            def emit_phase1_gp(gp):
                # k projections for chunks 2gp, 2gp+1 -> one FD1024 evac
                ep_k = epool.tile([128, 4, CHUNK], f32, tag="e", name="e")
                for i in range(4):
                    cs = slice((4 * gp + i) * CHUNK,
                               (4 * gp + i + 1) * CHUNK)
                    nc.tensor.matmul(ep_k[:, i, :], w2t4[:], x8[:, :, cs],
                                     start=True, stop=True, perf_mode=DR)
                # phase 1 is DVE-critical (exps + evacs): k evacs split
                # 2/2 and v evacs 5/3 toward the cheaper-per-op ACT
                if gp % 2 == 0:
                    nc.vector.tensor_scalar(
                        k_sb[:, 4 * gp:4 * gp + 4, :], ep_k[:],
                        1.0 / WSCALE, b2r4[:], MUL, ADD)
                else:
                    nc.scalar.activation(
                        k_sb[:, 4 * gp:4 * gp + 4, :], ep_k[:],
                        Identity, bias=b2r4[:], scale=1.0 / WSCALE)
                for i in range(2):
                    g = 2 * gp + i
                    # v for the 4 j-tiles of chunk g: one FD1024 evac
                    ep_v = epool.tile([128, 4, CHUNK], f32, tag="e",
                                      name="e")
                    for jj in range(4):
                        j = 4 * g + jj
                        nc.tensor.matmul(
                            ep_v[:, jj, :],
                            x8[:, :, j * 128:(j + 1) * 128], w3t[:],
                            start=True, stop=True, perf_mode=DR)
                    dst = vt[:, 4 * g:4 * g + 4, 0:C]
                    if g % 2 == 0 or g == 7:
                        nc.scalar.activation(dst, ep_v[:], Copy,
                                             scale=1.0 / WSCALE)
                    else:
                        nc.vector.tensor_scalar(dst, ep_v[:], 1.0 / WSCALE,
                                                None, MUL)
                    # sweep-0 quad for key tiles 4g..4g+3 vs query chunk 0
                    emit_quad(0, g)

            # ---- sweeps 1..16: per u (j-quad): one quad + exp for sweep
            # s, and 4 AV matmuls for sweep s-1. The AVs of u 0..3 cover
            # i-tile 0, u 4..7 i-tile 1 (staggered po lifetimes so the
            # 2-buf po pool never reallocates before its flush is
            # emitted). AV first: its pt inputs are a sweep old and
            # always ready, so the PE covers the quad's ep-buffer wait
            # with useful work.
            po = None
            pending = None     # (po, i0) awaiting evacuation; deferred to
            # the next it-group so the copy never head-of-line-blocks the
            # exp stream on either engine queue

            def flush_pending():
                nonlocal pending
                if pending is None:
                    return
                fpo, fi0 = pending
                ot = iopool.tile([128, 257], bf16, tag="ot", name="ot")
                if state["nflush"] % 2 == 0:
                    nc.scalar.activation(ot[:], fpo[:], Copy)
                else:
                    nc.vector.tensor_copy(ot[:], fpo[:])
                state["nflush"] += 1
                nc.gpsimd.dma_start(outt_d[fi0:fi0 + 128, :], ot[:])
                pending = None

            def emit_av(bs, it, kk):
                up, p = kk // 2, kk % 2
                nc.tensor.matmul(
                    po[:],
                    pt_handles[bs][up][:, 2 * p:2 * p + 2,
                                       it * 128:(it + 1) * 128],
                    vt[:, 2 * kk:2 * kk + 2, 0:257],
                    start=(kk == 0), stop=(kk == 15),
                    perf_mode=DR)

            # Pairing two quads per AV block halves the full-array<->pack
            # mode switches (each costs ~200ns of PE pipeline drain); the
            # two quads of a pair use different row-group sets AND banks,
            # so their strips overlap up to 4-wide.
            acnt = {}

            def emit_av_block(bs, n, qp=None):
                nonlocal po, pending
                for _ in range(n):
                    a = acnt.get(bs, 0)
                    acnt[bs] = a + 1
                    it, kk = a // 16, a % 16
                    if kk == 0:
                        flush_pending()
                        po = opool.tile([128, 257], f32, tag="o",
                                        name="o")
                        if it == 0 and qp is not None:
                            emit_qproj(qp)
                    emit_av(bs, it, kk)
                    if kk == 15:
                        pending = (po, bs * CHUNK + it * 128)

            # ---- phase 1: projections + sweep-0 quads, then sweeps.
            emit_qproj(0)
            for gp in range(4):
                emit_phase1_gp(gp)

            # ---- sweeps 1..16. Quads are grouped into trains of {3,3,2}
            # per sweep: each full-array<->pack transition costs ~197ns of
            # PE pipeline drain (row-conflict, measured; dtype/FWL-
            # invariant) while strip->strip issue inside a train is ~3ns,
            # so 3 transitions per sweep beat 8. Trains go BEFORE their AV
            # blocks so the exps enqueue early and ep buffers recycle with
            # slack.
            groups = [(3, 16), (3, 12), (2, 4)]
            for s in range(1, ICH + 1):
                bs = s - 1
                qbase = 0
                for gi, (nq, nav) in enumerate(groups):
                    if s < ICH:
                        for j in range(nq):
                            emit_quad(s, qbase + j)
                        qbase += nq
                    qp = None
                    if gi == 0 and s % 2 == 1 and s + 2 < ICH:
                        qp = s + 1
                    emit_av_block(bs, nav, qp=qp)
            flush_pending()

    nc.compile()
    return nc


def _get_program():
    if "nc" not in _cache:
        _cache["nc"] = _build_program()
    return _cache["nc"]


def _in_maps(inputs: dict) -> list:
    e4 = ml_dtypes.float8_e4m3
    x = np.asarray(inputs["x"], np.float32)
    w1 = np.asarray(inputs["w1"], np.float32)
    w2 = np.asarray(inputs["w2"], np.float32)
    w3 = np.asarray(inputs["w3"], np.float32)
    b1 = np.asarray(inputs["b1"], np.float32)
    b2 = np.asarray(inputs["b2"], np.float32)

    def rep4(w):  # [32, 256] -> [128, 2, 128] stationary, out cols tiled 4x
        wr = np.tile(w * WSCALE, (4, 1))                     # [128, 256]
        return np.ascontiguousarray(
            wr.T.reshape(2, 128, 128).transpose(1, 0, 2)).astype(e4)

    w1t4 = rep4(w1)
    w2t4 = rep4(w2)
    w3t8 = np.ascontiguousarray(
        (w3 * WSCALE).T.reshape(2, 128, C).transpose(1, 0, 2)).astype(e4)
    b1r4 = np.tile(b1, 4)[:, None].astype(np.float32)
    b2r4 = np.tile(b2, 4)[:, None].astype(np.float32)
    maps = []
    for b in range(B):
        xb = x[b].reshape(C, HW)
        x8 = np.ascontiguousarray(
            xb.reshape(2, 128, HW).transpose(1, 0, 2)).astype(e4)
        maps.append({
            "x8": x8,
            "w1t4": w1t4, "w2t4": w2t4, "w3t": w3t8,
            "b1r4": b1r4, "b2r4": b2r4,
        })
    return maps


def kernel(**inputs) -> np.ndarray:
    from concourse.bass_utils import run_bass_kernel_spmd

    nc = _get_program()
    res = run_bass_kernel_spmd(nc, _in_maps(inputs), list(range(NCORES)))
    x = np.asarray(inputs["x"], np.float32)
    b3 = np.asarray(inputs["b3"], np.float32)
    out = np.empty((B, C, H, W), np.float32)
    for b in range(B):
        acc = res.results[b]["outt"].astype(np.float32)   # [HW, 257]
        attn_t = acc[:, 0:256] / acc[:, 256:257] + b3[None, :]
        out[b] = attn_t.T.reshape(C, H, W) + x[b]
    return out


# revision 45
# speedup vs baseline: 1.1932x; 1.0177x over previous
"""Trainium2 Bass kernel for a spatial self-attention block.

Reference computation (per batch element b):
    q = w1 @ x + b1   [32, HW]      (1x1 conv == channel-wise linear)
    k = w2 @ x + b2   [32, HW]
    v = w3 @ x + b3   [256, HW]
    e[i, j] = sum_c q[c, i] k[c, j]
    attn = softmax(e, axis=j)
    out[c, i] = sum_j v[c, j] attn[i, j] + x[c, i]

Sharding: batch (8) across the 8 NeuronCores, one image per core.

v2 design (vs the v1 pair-staggered kernel; ~131us vs v1's ~143us at
the full-clock DVFS epoch; the shared host also has a ~1.2x-slower
epoch, run-level, visible as AV matmul dur 327ns vs 273ns):
  * Sweeps of 256 queries (16 sweeps). Each j-quad (4 key tiles x 256
    queries) is 4 pack matmuls into ONE [128,4,256] 2-bank PSUM tile
    consumed by ONE exp slot (FD=1024) - a single release event per
    quad. PSUM: 3 quad tiles (6 banks) + 2 AV accumulators = 8 banks.
  * HW constraint found by bisection: two CONCURRENT pack strips
    streaming into the same PSUM bank wedge the device (redacted
    INTERNAL at runtime; CoreSim doesn't model it). Strips sharing a
    bank therefore share a PE row-group (the row band serializes
    them); bank leaders run 2-wide, and alternating quads use the
    other row-group pair so adjacent quads can overlap 4-wide.
  * Measured transition tax (microbench, dtype/FWL-invariant):
    full-array<->pack costs ~197ns of PE pipeline drain per switch;
    strip->strip issue is ~3ns. Quads are therefore grouped in trains
    of {3,3,2} per sweep ahead of AV blocks of {16,12,4}: the longer
    first block lets the exp engines drain so the next train's ep
    buffers release together and the train actually forms. Mid-sweep
    the ep-release pacing (one FD1024 exp per ~585ns across ACT+DVE)
    still degrades some trains to solo quads - structural with 3 ep
    buffers; 4 don't fit in PSUM.
  * Exp slots strictly alternate DVE (Schraudolph fast-exp
    tensor_scalar into a u8-viewed tile) / ACT (real Exp, fp8e4 out).
    Both are 1x-capped by the f32 PSUM read port - the exp wall is
    ~90us/engine including evacs+flushes and is the co-floor with the
    PE (~116us busy).
  * Evacuations batched to amortize the ~150-230ns per-op fixed cost:
    v FD=1024 (one per 512-col chunk), k FD=1024 (per chunk pair),
    q FD=512 (per sweep pair). Input DMAs split across the sync and
    gpsimd queues so the phase-1 load pipeline is ~2x wider.
  * AV: fp8 DoubleRow, stationary = pt strips (K=256 keys), moving =
    vt[.,257] (v channels | ones column -> softmax denominator),
    N=257 at ~110ns issue rate (1 col/cycle @2.4GHz; DR doubles K per
    instruction, not the column rate); raw accumulator flushed bf16
    to HBM, normalization + residual on host (error budget: attention
    output is ~170x smaller in norm than the residual).
  * Weights pre-scaled by 64 (pow2) for fp8 range; 1/64 folded into
    the PSUM evacuations. End-to-end rel err ~7e-4 (gate 2e-2).
"""

import numpy as np
import ml_dtypes

B, C, H, W = 8, 256, 64, 64
HW = H * W          # 4096
CQK = C // 8        # 32
NCORES = 8
NJ = HW // 128      # 32 key tiles
ICH = 16            # query-dim chunks (sweeps)
CHUNK = HW // ICH   # 256 queries per sweep
NQ = NJ // 4        # 8 quads of key tiles per sweep
VSTRIDE = 260       # vT free-dim stride per j-tile (257 used, 260 align)
WSCALE = 64.0       # host-side pow2 weight prescale for fp8 range

# Schraudolph fast-exp constants (e4m3 bit pattern): bits = A8*e + B8
A8 = 8.0 * 1.4426950408889634   # 8 * log2(e)
B8 = 56.0 - 0.45                # (bias 7)*8 minus tuned PWL offset

_cache: dict = {}


def _build_program():
    import concourse.bacc as bacc
    import concourse.mybir as mybir
    import concourse.tile as tile

    f32 = mybir.dt.float32
    bf16 = mybir.dt.bfloat16
    fp8 = mybir.dt.float8e4
    u8 = mybir.dt.uint8
    Exp = mybir.ActivationFunctionType.Exp
    Identity = mybir.ActivationFunctionType.Identity
    Copy = mybir.ActivationFunctionType.Copy
    DR = mybir.MatmulPerfMode.DoubleRow
    MUL = mybir.AluOpType.mult
    ADD = mybir.AluOpType.add

    nc = bacc.Bacc(None)
    x8_d = nc.dram_tensor("x8", [128, 2, HW], fp8, kind="ExternalInput")
    w1_d = nc.dram_tensor("w1t4", [128, 2, 128], fp8, kind="ExternalInput")
    w2_d = nc.dram_tensor("w2t4", [128, 2, 128], fp8, kind="ExternalInput")
    w3_d = nc.dram_tensor("w3t", [128, 2, C], fp8, kind="ExternalInput")
    b1_d = nc.dram_tensor("b1r4", [128, 1], f32, kind="ExternalInput")
    b2_d = nc.dram_tensor("b2r4", [128, 1], f32, kind="ExternalInput")
    # raw AV accumulator (numerator | denominator): normalization and the
    # +x residual happen on the host in fp32
    outt_d = nc.dram_tensor("outt", [HW, 257], bf16, kind="ExternalOutput")

    with tile.TileContext(nc) as tc:
        with (
            tc.tile_pool(name="const", bufs=1) as cpool,
            tc.tile_pool(name="xin", bufs=1) as xpool,
            tc.tile_pool(name="qk", bufs=1) as qkpool,
            tc.tile_pool(name="pt", bufs=18) as ptpool,
            tc.tile_pool(name="io", bufs=3) as iopool,
            tc.tile_pool(name="psume", bufs=3, space="PSUM") as epool,
            tc.tile_pool(name="psumo", bufs=2, space="PSUM") as opool,
        ):
            # ---- constants / weights ----
            w1t4 = cpool.tile([128, 2, 128], fp8, tag="w1t4", name="w1t4")
            w2t4 = cpool.tile([128, 2, 128], fp8, tag="w2t4", name="w2t4")
            w3t = cpool.tile([128, 2, C], fp8, tag="w3t", name="w3t")
            b1r4 = cpool.tile([128, 1], f32, tag="b1r4", name="b1r4")
            b2r4 = cpool.tile([128, 1], f32, tag="b2r4", name="b2r4")
            x8 = xpool.tile([128, 2, HW], fp8, tag="x8", name="x8")
            # Inputs split across the sync and (phase-1-idle) gpsimd DMA
            # queues so the load pipeline is ~2x wider; x chunk 0 + the
            # small qk weights lead so the first kproj can start ASAP.
            nc.sync.dma_start(w2t4[:], w2_d[:])
            nc.sync.dma_start(x8[:, :, 0:256], x8_d[:, :, 0:256])
            nc.sync.dma_start(x8[:, :, 256:512], x8_d[:, :, 256:512])
            nc.sync.dma_start(w1t4[:], w1_d[:])
            nc.sync.dma_start(b2r4[:], b2_d[:])
            nc.sync.dma_start(b1r4[:], b1_d[:])
            nc.sync.dma_start(w3t[:], w3_d[:])
            for g in range(1, 8):
                cs = slice(g * 512, (g + 1) * 512)
                nc.gpsimd.dma_start(x8[:, :, cs], x8_d[:, :, cs])

            warm = cpool.tile([128, 512], bf16, tag="warm", name="warm")
            nc.vector.memset(warm[:], 0.0)

            # q and k live replicated 4x along the partition axis (copies
            # at base partitions 0/32/64/96) so the eT pack matmuls can use
            # all 128 PE rows. vt[j, c] holds v (plus a ones column at 256
            # that makes the softmax denominator fall out of the AV pass).
            q_sb = qkpool.tile([128, ICH, CHUNK], bf16, tag="q", name="q")
            k_sb = qkpool.tile([128, ICH, CHUNK], bf16, tag="k", name="k")
            vt = qkpool.tile([128, NJ, VSTRIDE], fp8, tag="vt", name="vt")
            nc.vector.memset(vt[:, :, 256:257], 1.0)

            def warm_mm(dst, n=1):
                # dummy full-array matmul into a psum region that a real
                # matmul overwrites later (start=True resets it); trips the
                # HAM activity monitor so the PE clock ramps to 2.4 GHz
                for _ in range(n):
                    nc.tensor.matmul(dst, warm[:, 0:128], warm[:, 0:256],
                                     start=True, stop=True)

            # global exp-slot parity: strict ACT/DVE alternation
            state = {"slot": 0, "nflush": 0}
            pt_handles = [[None] * NQ for _ in range(ICH)]

            def emit_exp(ep, s, u):
                pt = ptpool.tile([128, 4, CHUNK], fp8, tag="pt", name="pt")
                if state["slot"] % 2 == 0:
                    nc.vector.tensor_scalar(pt.bitcast(u8)[:], ep[:],
                                            A8, B8, MUL, ADD)
                else:
                    nc.scalar.activation(pt[:], ep[:], Exp)
                state["slot"] += 1
                pt_handles[s][u] = pt

            def emit_quad(s, u):
                # 4 pack matmuls -> one 2-bank tile. HW constraint (found
                # by bisection): two CONCURRENT pack strips into the same
                # PSUM bank wedge the device. Strips sharing a bank
                # therefore share a PE row-group (the row band serializes
                # them); the two banks' leaders run concurrently, and
                # alternating quads use the other row-group pair so
                # consecutive quads can overlap 4-wide.
                ep = epool.tile([128, 4, CHUNK], f32, tag="e", name="e")
                pb = (u % 2) * 2
                for i in (0, 2, 1, 3):   # bank leaders first, then trailers
                    jt = 4 * u + i
                    t = pb + i // 2
                    nc.tensor.matmul(
                        ep[:, i, :],
                        k_sb[t * CQK:(t + 1) * CQK, jt // 2,
                             (jt % 2) * 128:(jt % 2) * 128 + 128],
                        q_sb[t * CQK:(t + 1) * CQK, s, :],
                        start=True, stop=True,
                        tile_position=(t * CQK, 0))
                emit_exp(ep, s, u)

            def emit_qproj(s):
                # q columns for sweeps s, s+1 (512 cols), one evac on ACT
                ep_q = epool.tile([128, 4, CHUNK], f32, tag="e", name="e")
                for i in range(2):
                    cs = slice((s + i) * CHUNK, (s + i + 1) * CHUNK)
                    nc.tensor.matmul(ep_q[:, i, :], w1t4[:], x8[:, :, cs],
                                     start=True, stop=True, perf_mode=DR)
                nc.scalar.activation(
                    q_sb[:, s:s + 2, :], ep_q[:, 0:2, :],
                    Identity, bias=b1r4[:], scale=1.0 / WSCALE)

            wep = epool.tile([128, 4, CHUNK], f32, tag="e", name="e")
            warm_mm(wep[:, 0, :], 2)

            def emit_phase1_gp(gp):
                # k projections for chunks 2gp, 2gp+1 -> one FD1024 evac
                ep_k = epool.tile([128, 4, CHUNK], f32, tag="e", name="e")
                for i in range(4):
                    cs = slice((4 * gp + i) * CHUNK,
                               (4 * gp + i + 1) * CHUNK)
                    nc.tensor.matmul(ep_k[:, i, :], w2t4[:], x8[:, :, cs],
                                     start=True, stop=True, perf_mode=DR)
                # phase 1 is DVE-critical (exps + evacs): k evacs split
                # 2/2 and v evacs 5/3 toward the cheaper-per-op ACT
                if gp % 2 == 0:
                    nc.vector.tensor_scalar(
                        k_sb[:, 4 * gp:4 * gp + 4, :], ep_k[:],
                        1.0 / WSCALE, b2r4[:], MUL, ADD)
                else:
                    nc.scalar.activation(
                        k_sb[:, 4 * gp:4 * gp + 4, :], ep_k[:],
                        Identity, bias=b2r4[:], scale=1.0 / WSCALE)
                for i in range(2):
                    g = 2 * gp + i
                    # v for the 4 j-tiles of chunk g: one FD1024 evac
                    ep_v = epool.tile([128, 4, CHUNK], f32, tag="e",
                                      name="e")
                    for jj in range(4):
                        j = 4 * g + jj
                        nc.tensor.matmul(
                            ep_v[:, jj, :],
                            x8[:, :, j * 128:(j + 1) * 128], w3t[:],
                            start=True, stop=True, perf_mode=DR)
                    dst = vt[:, 4 * g:4 * g + 4, 0:C]
                    if g % 2 == 0 or g == 7:
                        nc.scalar.activation(dst, ep_v[:], Copy,
                                             scale=1.0 / WSCALE)
                    else:
                        nc.vector.tensor_scalar(dst, ep_v[:], 1.0 / WSCALE,
                                                None, MUL)
                    # sweep-0 quad for key tiles 4g..4g+3 vs query chunk 0
                    emit_quad(0, g)

            # ---- sweeps 1..16: per u (j-quad): one quad + exp for sweep
            # s, and 4 AV matmuls for sweep s-1. The AVs of u 0..3 cover
            # i-tile 0, u 4..7 i-tile 1 (staggered po lifetimes so the
            # 2-buf po pool never reallocates before its flush is
            # emitted). AV first: its pt inputs are a sweep old and
            # always ready, so the PE covers the quad's ep-buffer wait
            # with useful work.
            po = None
            pending = None     # (po, i0) awaiting evacuation; deferred to
            # the next it-group so the copy never head-of-line-blocks the
            # exp stream on either engine queue

            def flush_pending():
                nonlocal pending
                if pending is None:
                    return
                fpo, fi0 = pending
                ot = iopool.tile([128, 257], bf16, tag="ot", name="ot")
                if state["nflush"] % 2 == 0:
                    nc.scalar.activation(ot[:], fpo[:], Copy)
                else:
                    nc.vector.tensor_copy(ot[:], fpo[:])
                state["nflush"] += 1
                nc.gpsimd.dma_start(outt_d[fi0:fi0 + 128, :], ot[:])
                pending = None

            def emit_av(bs, it, kk):
                up, p = kk // 2, kk % 2
                nc.tensor.matmul(
                    po[:],
                    pt_handles[bs][up][:, 2 * p:2 * p + 2,
                                       it * 128:(it + 1) * 128],
                    vt[:, 2 * kk:2 * kk + 2, 0:257],
                    start=(kk == 0), stop=(kk == 15),
                    perf_mode=DR)

            # Pairing two quads per AV block halves the full-array<->pack
            # mode switches (each costs ~200ns of PE pipeline drain); the
            # two quads of a pair use different row-group sets AND banks,
            # so their strips overlap up to 4-wide.
            acnt = {}

            def emit_av_block(bs, n, qp=None):
                nonlocal po, pending
                for _ in range(n):
                    a = acnt.get(bs, 0)
                    acnt[bs] = a + 1
                    it, kk = a // 16, a % 16
                    if kk == 0:
                        flush_pending()
                        po = opool.tile([128, 257], f32, tag="o",
                                        name="o")
                        if it == 0 and qp is not None:
                            emit_qproj(qp)
                    emit_av(bs, it, kk)
                    if kk == 15:
                        pending = (po, bs * CHUNK + it * 128)

            # ---- phase 1: projections + sweep-0 quads, then sweeps.
            emit_qproj(0)
            for gp in range(4):
                emit_phase1_gp(gp)

            # ---- sweeps 1..16. Quads are grouped into trains of {3,3,2}
            # per sweep: each full-array<->pack transition costs ~197ns of
            # PE pipeline drain (row-conflict, measured; dtype/FWL-
            # invariant) while strip->strip issue inside a train is ~3ns,
            # so 3 transitions per sweep beat 8. Trains go BEFORE their AV
            # blocks so the exps enqueue early and ep buffers recycle with
            # slack.
            groups = [(3, 16), (3, 12), (2, 4)]
            for s in range(1, ICH + 1):
                bs = s - 1
                qbase = 0
                for gi, (nq, nav) in enumerate(groups):
                    if s < ICH:
                        for j in range(nq):
                            emit_quad(s, qbase + j)
                        qbase += nq
                    qp = None
                    if gi == 0 and s % 2 == 1 and s + 2 < ICH:
                        qp = s + 1
                    emit_av_block(bs, nav, qp=qp)
            flush_pending()

    nc.compile()
    return nc


def _get_program():
    if "nc" not in _cache:
        _cache["nc"] = _build_program()
    return _cache["nc"]


def _in_maps(inputs: dict) -> list:
    e4 = ml_dtypes.float8_e4m3
    x = np.asarray(inputs["x"], np.float32)
    w1 = np.asarray(inputs["w1"], np.float32)
    w2 = np.asarray(inputs["w2"], np.float32)
    w3 = np.asarray(inputs["w3"], np.float32)
    b1 = np.asarray(inputs["b1"], np.float32)
    b2 = np.asarray(inputs["b2"], np.float32)

    def rep4(w):  # [32, 256] -> [128, 2, 128] stationary, out cols tiled 4x
        wr = np.tile(w * WSCALE, (4, 1))                     # [128, 256]
        return np.ascontiguousarray(
            wr.T.reshape(2, 128, 128).transpose(1, 0, 2)).astype(e4)

    w1t4 = rep4(w1)
    w2t4 = rep4(w2)
    w3t8 = np.ascontiguousarray(
        (w3 * WSCALE).T.reshape(2, 128, C).transpose(1, 0, 2)).astype(e4)
    b1r4 = np.tile(b1, 4)[:, None].astype(np.float32)
    b2r4 = np.tile(b2, 4)[:, None].astype(np.float32)
    maps = []
    for b in range(B):
        xb = x[b].reshape(C, HW)
        x8 = np.ascontiguousarray(
            xb.reshape(2, 128, HW).transpose(1, 0, 2)).astype(e4)
        maps.append({
            "x8": x8,
            "w1t4": w1t4, "w2t4": w2t4, "w3t": w3t8,
            "b1r4": b1r4, "b2r4": b2r4,
        })
    return maps


def kernel(**inputs) -> np.ndarray:
    from concourse.bass_utils import run_bass_kernel_spmd

    nc = _get_program()
    res = run_bass_kernel_spmd(nc, _in_maps(inputs), list(range(NCORES)))
    x = np.asarray(inputs["x"], np.float32)
    b3 = np.asarray(inputs["b3"], np.float32)
    out = np.empty((B, C, H, W), np.float32)
    for b in range(B):
        acc = res.results[b]["outt"].astype(np.float32)   # [HW, 257]
        attn_t = acc[:, 0:256] / acc[:, 256:257] + b3[None, :]
        out[b] = attn_t.T.reshape(C, H, W) + x[b]
    return out


# revision 46
# speedup vs baseline: 1.2096x; 1.0137x over previous
"""Trainium2 Bass kernel for a spatial self-attention block.

Reference computation (per batch element b):
    q = w1 @ x + b1   [32, HW]      (1x1 conv == channel-wise linear)
    k = w2 @ x + b2   [32, HW]
    v = w3 @ x + b3   [256, HW]
    e[i, j] = sum_c q[c, i] k[c, j]
    attn = softmax(e, axis=j)
    out[c, i] = sum_j v[c, j] attn[i, j] + x[c, i]

Sharding: batch (8) across the 8 NeuronCores, one image per core.

v2 design (vs the v1 pair-staggered kernel; ~131us vs v1's ~143us at
the full-clock DVFS epoch; the shared host also has a ~1.2x-slower
epoch, run-level, visible as AV matmul dur 327ns vs 273ns):
  * Sweeps of 256 queries (16 sweeps). Each j-quad (4 key tiles x 256
    queries) is 4 pack matmuls into ONE [128,4,256] 2-bank PSUM tile
    consumed by ONE exp slot (FD=1024) - a single release event per
    quad. PSUM: 3 quad tiles (6 banks) + 2 AV accumulators = 8 banks.
  * HW constraint found by bisection: two CONCURRENT pack strips
    streaming into the same PSUM bank wedge the device (redacted
    INTERNAL at runtime; CoreSim doesn't model it). Strips sharing a
    bank therefore share a PE row-group (the row band serializes
    them); bank leaders run 2-wide, and alternating quads use the
    other row-group pair so adjacent quads can overlap 4-wide.
  * Measured transition tax (microbench, dtype/FWL-invariant):
    full-array<->pack costs ~197ns of PE pipeline drain per switch;
    strip->strip issue is ~3ns. Quads are therefore grouped in trains
    of {3,3,2} per sweep ahead of AV blocks of {16,12,4}: the longer
    first block lets the exp engines drain so the next train's ep
    buffers release together and the train actually forms. Mid-sweep
    the ep-release pacing (one FD1024 exp per ~585ns across ACT+DVE)
    still degrades some trains to solo quads - structural with 3 ep
    buffers; 4 don't fit in PSUM.
  * Exp slots strictly alternate DVE (Schraudolph fast-exp
    tensor_scalar into a u8-viewed tile) / ACT (real Exp, fp8e4 out).
    Both are 1x-capped by the f32 PSUM read port - the exp wall is
    ~90us/engine including evacs+flushes and is the co-floor with the
    PE (~116us busy).
  * Evacuations batched to amortize the ~150-230ns per-op fixed cost:
    v FD=1024 (one per 512-col chunk), k FD=1024 (per chunk pair),
    q FD=512 (per sweep pair). Input DMAs split across the sync and
    gpsimd queues so the phase-1 load pipeline is ~2x wider.
  * AV: fp8 DoubleRow, stationary = pt strips (K=256 keys), moving =
    vt[.,257] (v channels | ones column -> softmax denominator),
    N=257 at ~110ns issue rate (1 col/cycle @2.4GHz; DR doubles K per
    instruction, not the column rate); raw accumulator flushed bf16
    to HBM, normalization + residual on host (error budget: attention
    output is ~170x smaller in norm than the residual).
  * Weights pre-scaled by 64 (pow2) for fp8 range; 1/64 folded into
    the PSUM evacuations. End-to-end rel err ~7e-4 (gate 2e-2).
"""

import numpy as np
import ml_dtypes

B, C, H, W = 8, 256, 64, 64
HW = H * W          # 4096
CQK = C // 8        # 32
NCORES = 8
NJ = HW // 128      # 32 key tiles
ICH = 16            # query-dim chunks (sweeps)
CHUNK = HW // ICH   # 256 queries per sweep
NQ = NJ // 4        # 8 quads of key tiles per sweep
VSTRIDE = 260       # vT free-dim stride per j-tile (257 used, 260 align)
WSCALE = 64.0       # host-side pow2 weight prescale for fp8 range

# Schraudolph fast-exp constants (e4m3 bit pattern): bits = A8*e + B8
A8 = 8.0 * 1.4426950408889634   # 8 * log2(e)
B8 = 56.0 - 0.45                # (bias 7)*8 minus tuned PWL offset

_cache: dict = {}


def _build_program():
    import concourse.bacc as bacc
    import concourse.mybir as mybir
    import concourse.tile as tile

    f32 = mybir.dt.float32
    bf16 = mybir.dt.bfloat16
    fp8 = mybir.dt.float8e4
    u8 = mybir.dt.uint8
    Exp = mybir.ActivationFunctionType.Exp
    Identity = mybir.ActivationFunctionType.Identity
    Copy = mybir.ActivationFunctionType.Copy
    DR = mybir.MatmulPerfMode.DoubleRow
    MUL = mybir.AluOpType.mult
    ADD = mybir.AluOpType.add

    nc = bacc.Bacc(None)
    x8_d = nc.dram_tensor("x8", [128, 2, HW], fp8, kind="ExternalInput")
    wqk_d = nc.dram_tensor("wqk", [128, 2, 256], fp8, kind="ExternalInput")
    w3_d = nc.dram_tensor("w3t", [128, 2, C], fp8, kind="ExternalInput")
    b12_d = nc.dram_tensor("b12", [128, 2], f32, kind="ExternalInput")
    # raw AV accumulator (numerator | denominator): normalization and the
    # +x residual happen on the host in fp32
    outt_d = nc.dram_tensor("outt", [HW, 257], bf16, kind="ExternalOutput")

    with tile.TileContext(nc) as tc:
        with (
            tc.tile_pool(name="const", bufs=1) as cpool,
            tc.tile_pool(name="xin", bufs=1) as xpool,
            tc.tile_pool(name="qk", bufs=1) as qkpool,
            tc.tile_pool(name="pt", bufs=18) as ptpool,
            tc.tile_pool(name="io", bufs=3) as iopool,
            tc.tile_pool(name="psume", bufs=3, space="PSUM") as epool,
            tc.tile_pool(name="psumo", bufs=2, space="PSUM") as opool,
        ):
            # ---- constants / weights ----
            wqk = cpool.tile([128, 2, 256], fp8, tag="wqk", name="wqk")
            w2t4 = wqk[:, :, 0:128]
            w1t4 = wqk[:, :, 128:256]
            w3t = cpool.tile([128, 2, C], fp8, tag="w3t", name="w3t")
            b12 = cpool.tile([128, 2], f32, tag="b12", name="b12")
            b2r4 = b12[:, 0:1]
            b1r4 = b12[:, 1:2]
            x8 = xpool.tile([128, 2, HW], fp8, tag="x8", name="x8")
            # Inputs split across the sync and (phase-1-idle) gpsimd DMA
            # queues so the load pipeline is ~2x wider; x chunk 0 + the
            # small qk weights lead so the first kproj can start ASAP.
            nc.sync.dma_start(wqk[:], wqk_d[:])
            nc.sync.dma_start(x8[:, :, 0:256], x8_d[:, :, 0:256])
            nc.sync.dma_start(x8[:, :, 256:512], x8_d[:, :, 256:512])
            nc.sync.dma_start(b12[:], b12_d[:])
            nc.sync.dma_start(w3t[:], w3_d[:])
            for g in range(1, 8):
                cs = slice(g * 512, (g + 1) * 512)
                nc.gpsimd.dma_start(x8[:, :, cs], x8_d[:, :, cs])

            warm = cpool.tile([128, 512], bf16, tag="warm", name="warm")
            nc.vector.memset(warm[:], 0.0)

            # q and k live replicated 4x along the partition axis (copies
            # at base partitions 0/32/64/96) so the eT pack matmuls can use
            # all 128 PE rows. vt[j, c] holds v (plus a ones column at 256
            # that makes the softmax denominator fall out of the AV pass).
            q_sb = qkpool.tile([128, ICH, CHUNK], bf16, tag="q", name="q")
            k_sb = qkpool.tile([128, ICH, CHUNK], bf16, tag="k", name="k")
            vt = qkpool.tile([128, NJ, VSTRIDE], fp8, tag="vt", name="vt")
            nc.vector.memset(vt[:, :, 256:257], 1.0)

            def warm_mm(dst, n=1):
                # dummy full-array matmul into a psum region that a real
                # matmul overwrites later (start=True resets it); trips the
                # HAM activity monitor so the PE clock ramps to 2.4 GHz
                for _ in range(n):
                    nc.tensor.matmul(dst, warm[:, 0:128], warm[:, 0:256],
                                     start=True, stop=True)

            # global exp-slot parity: strict ACT/DVE alternation
            state = {"slot": 0, "nflush": 0}
            pt_handles = [[None] * NQ for _ in range(ICH)]

            def emit_exp(ep, s, u):
                pt = ptpool.tile([128, 4, CHUNK], fp8, tag="pt", name="pt")
                if state["slot"] % 2 == 0:
                    nc.vector.tensor_scalar(pt.bitcast(u8)[:], ep[:],
                                            A8, B8, MUL, ADD)
                else:
                    nc.scalar.activation(pt[:], ep[:], Exp)
                state["slot"] += 1
                pt_handles[s][u] = pt

            def emit_quad(s, u):
                # 4 pack matmuls -> one 2-bank tile. HW constraint (found
                # by bisection): two CONCURRENT pack strips into the same
                # PSUM bank wedge the device. Strips sharing a bank
                # therefore share a PE row-group (the row band serializes
                # them); the two banks' leaders run concurrently, and
                # alternating quads use the other row-group pair so
                # consecutive quads can overlap 4-wide.
                ep = epool.tile([128, 4, CHUNK], f32, tag="e", name="e")
                pb = (u % 2) * 2
                for i in (0, 2, 1, 3):   # bank leaders first, then trailers
                    jt = 4 * u + i
                    t = pb + i // 2
                    nc.tensor.matmul(
                        ep[:, i, :],
                        k_sb[t * CQK:(t + 1) * CQK, jt // 2,
                             (jt % 2) * 128:(jt % 2) * 128 + 128],
                        q_sb[t * CQK:(t + 1) * CQK, s, :],
                        start=True, stop=True,
                        tile_position=(t * CQK, 0))
                emit_exp(ep, s, u)

            def emit_qproj(s):
                # q columns for sweeps s, s+1 (512 cols), one evac on ACT
                ep_q = epool.tile([128, 4, CHUNK], f32, tag="e", name="e")
                for i in range(2):
                    cs = slice((s + i) * CHUNK, (s + i + 1) * CHUNK)
                    nc.tensor.matmul(ep_q[:, i, :], w1t4, x8[:, :, cs],
                                     start=True, stop=True, perf_mode=DR)
                nc.scalar.activation(
                    q_sb[:, s:s + 2, :], ep_q[:, 0:2, :],
                    Identity, bias=b1r4, scale=1.0 / WSCALE)

            wep = epool.tile([128, 4, CHUNK], f32, tag="e", name="e")
            warm_mm(wep[:, 0, :], 2)

            def emit_phase1_gp(gp):
                # k projections for chunks 2gp, 2gp+1 -> one FD1024 evac
                ep_k = epool.tile([128, 4, CHUNK], f32, tag="e", name="e")
                for i in range(4):
                    cs = slice((4 * gp + i) * CHUNK,
                               (4 * gp + i + 1) * CHUNK)
                    nc.tensor.matmul(ep_k[:, i, :], w2t4, x8[:, :, cs],
                                     start=True, stop=True, perf_mode=DR)
                # phase 1 is DVE-critical (exps + evacs): k evacs split
                # 2/2 and v evacs 5/3 toward the cheaper-per-op ACT
                if gp % 2 == 0:
                    nc.vector.tensor_scalar(
                        k_sb[:, 4 * gp:4 * gp + 4, :], ep_k[:],
                        1.0 / WSCALE, b2r4, MUL, ADD)
                else:
                    nc.scalar.activation(
                        k_sb[:, 4 * gp:4 * gp + 4, :], ep_k[:],
                        Identity, bias=b2r4, scale=1.0 / WSCALE)
                for i in range(2):
                    g = 2 * gp + i
                    # v for the 4 j-tiles of chunk g: one FD1024 evac
                    ep_v = epool.tile([128, 4, CHUNK], f32, tag="e",
                                      name="e")
                    for jj in range(4):
                        j = 4 * g + jj
                        nc.tensor.matmul(
                            ep_v[:, jj, :],
                            x8[:, :, j * 128:(j + 1) * 128], w3t[:],
                            start=True, stop=True, perf_mode=DR)
                    dst = vt[:, 4 * g:4 * g + 4, 0:C]
                    if g % 2 == 0 or g == 7:
                        nc.scalar.activation(dst, ep_v[:], Copy,
                                             scale=1.0 / WSCALE)
                    else:
                        nc.vector.tensor_scalar(dst, ep_v[:], 1.0 / WSCALE,
                                                None, MUL)
                    # sweep-0 quad for key tiles 4g..4g+3 vs query chunk 0
                    emit_quad(0, g)

            # ---- sweeps 1..16: per u (j-quad): one quad + exp for sweep
            # s, and 4 AV matmuls for sweep s-1. The AVs of u 0..3 cover
            # i-tile 0, u 4..7 i-tile 1 (staggered po lifetimes so the
            # 2-buf po pool never reallocates before its flush is
            # emitted). AV first: its pt inputs are a sweep old and
            # always ready, so the PE covers the quad's ep-buffer wait
            # with useful work.
            po = None
            pending = None     # (po, i0) awaiting evacuation; deferred to
            # the next it-group so the copy never head-of-line-blocks the
            # exp stream on either engine queue

            def flush_pending():
                nonlocal pending
                if pending is None:
                    return
                fpo, fi0 = pending
                ot = iopool.tile([128, 257], bf16, tag="ot", name="ot")
                if state["nflush"] % 2 == 0:
                    nc.scalar.activation(ot[:], fpo[:], Copy)
                else:
                    nc.vector.tensor_copy(ot[:], fpo[:])
                state["nflush"] += 1
                nc.gpsimd.dma_start(outt_d[fi0:fi0 + 128, :], ot[:])
                pending = None

            def emit_av(bs, it, kk):
                up, p = kk // 2, kk % 2
                nc.tensor.matmul(
                    po[:],
                    pt_handles[bs][up][:, 2 * p:2 * p + 2,
                                       it * 128:(it + 1) * 128],
                    vt[:, 2 * kk:2 * kk + 2, 0:257],
                    start=(kk == 0), stop=(kk == 15),
                    perf_mode=DR)

            # Pairing two quads per AV block halves the full-array<->pack
            # mode switches (each costs ~200ns of PE pipeline drain); the
            # two quads of a pair use different row-group sets AND banks,
            # so their strips overlap up to 4-wide.
            acnt = {}

            def emit_av_block(bs, n, qp=None):
                nonlocal po, pending
                for _ in range(n):
                    a = acnt.get(bs, 0)
                    acnt[bs] = a + 1
                    it, kk = a // 16, a % 16
                    if kk == 0:
                        flush_pending()
                        po = opool.tile([128, 257], f32, tag="o",
                                        name="o")
                        if it == 0 and qp is not None:
                            emit_qproj(qp)
                    emit_av(bs, it, kk)
                    if kk == 15:
                        pending = (po, bs * CHUNK + it * 128)

            # ---- phase 1: projections + sweep-0 quads, then sweeps.
            emit_qproj(0)
            for gp in range(4):
                emit_phase1_gp(gp)

            # ---- sweeps 1..16. Quads are grouped into trains of {3,3,2}
            # per sweep: each full-array<->pack transition costs ~197ns of
            # PE pipeline drain (row-conflict, measured; dtype/FWL-
            # invariant) while strip->strip issue inside a train is ~3ns,
            # so 3 transitions per sweep beat 8. Trains go BEFORE their AV
            # blocks so the exps enqueue early and ep buffers recycle with
            # slack.
            groups = [(3, 16), (3, 12), (2, 4)]
            for s in range(1, ICH + 1):
                bs = s - 1
                qbase = 0
                for gi, (nq, nav) in enumerate(groups):
                    if s < ICH:
                        for j in range(nq):
                            emit_quad(s, qbase + j)
                        qbase += nq
                    qp = None
                    if gi == 0 and s % 2 == 1 and s + 2 < ICH:
                        qp = s + 1
                    emit_av_block(bs, nav, qp=qp)
            flush_pending()

    nc.compile()
    return nc


def _get_program():
    if "nc" not in _cache:
        _cache["nc"] = _build_program()
    return _cache["nc"]


def _in_maps(inputs: dict) -> list:
    e4 = ml_dtypes.float8_e4m3
    x = np.asarray(inputs["x"], np.float32)
    w1 = np.asarray(inputs["w1"], np.float32)
    w2 = np.asarray(inputs["w2"], np.float32)
    w3 = np.asarray(inputs["w3"], np.float32)
    b1 = np.asarray(inputs["b1"], np.float32)
    b2 = np.asarray(inputs["b2"], np.float32)

    def rep4(w):  # [32, 256] -> [128, 2, 128] stationary, out cols tiled 4x
        wr = np.tile(w * WSCALE, (4, 1))                     # [128, 256]
        return np.ascontiguousarray(
            wr.T.reshape(2, 128, 128).transpose(1, 0, 2)).astype(e4)

    wqk = np.concatenate([rep4(w2), rep4(w1)], axis=2)
    w3t8 = np.ascontiguousarray(
        (w3 * WSCALE).T.reshape(2, 128, C).transpose(1, 0, 2)).astype(e4)
    b12 = np.stack([np.tile(b2, 4), np.tile(b1, 4)],
                   axis=1).astype(np.float32)
    maps = []
    for b in range(B):
        xb = x[b].reshape(C, HW)
        x8 = np.ascontiguousarray(
            xb.reshape(2, 128, HW).transpose(1, 0, 2)).astype(e4)
        maps.append({
            "x8": x8,
            "wqk": wqk, "w3t": w3t8, "b12": b12,
        })
    return maps


def kernel(**inputs) -> np.ndarray:
    from concourse.bass_utils import run_bass_kernel_spmd

    nc = _get_program()
    res = run_bass_kernel_spmd(nc, _in_maps(inputs), list(range(NCORES)))
    x = np.asarray(inputs["x"], np.float32)
    b3 = np.asarray(inputs["b3"], np.float32)
    out = np.empty((B, C, H, W), np.float32)
    for b in range(B):
        acc = res.results[b]["outt"].astype(np.float32)   # [HW, 257]
        attn_t = acc[:, 0:256] / acc[:, 256:257] + b3[None, :]
        out[b] = attn_t.T.reshape(C, H, W) + x[b]
    return out
